# revision 20
# baseline (speedup 1.0000x reference)
"""Multi-head self-attention (B=4, T=2048, D=1024, H=16) on 8 Trainium2 cores.

Sharding: core c = 2*b + s owns batch b (of 4) and head-half s (heads
8s..8s+7).  Each core computes QKV + attention for its 8 heads in a
transposed layout and emits a [512-row, 2048] f16 partial of the output
projection; the host sums the two partials of each batch.

Attention layout per head pair (packed on SBUF partitions 0-63 / 64-127):
  S_T[tk, tq] = K_T.T @ Q_T   (two heads row-packed; the two 64-row
                               matmuls run concurrently on PE row tiles)
  P = exp(S_T / 8)            (ScalarE, scale folded into the activation)
  [O_T; denom] = [V | 1].T @ P_T   (ones column yields softmax denominators)

The inner loop is ScalarE(exp)-paced (~1.08us per key tile).  Emission is
software-pipelined one tile deep: tile t's PV pair (and any chunk
finalize) is emitted AFTER tile t+1's QK+exp, so the Tile scheduler's
priority heap always prefers the exp-critical QK pair when the PE goes
idle.  QKV projections for group g+1 drip-feed into group g's exp
bubbles; the output projection (all four group terms in one PSUM chain)
drips into group 3's bubbles chunk-by-chunk as soon as that chunk is
normalized.

kernel(x, w_qkv, w_proj) -> [4, 2048, 1024] float32
"""

import sys

sys.path.insert(0, "/opt/trn_rl_repo")

import numpy as np
import ml_dtypes

import concourse.bass as bass
import concourse.bacc as bacc
import concourse.mybir as mybir
import concourse.tile as tile
from concourse.bass_utils import run_bass_kernel_spmd
from concourse.masks import make_identity

BF16 = mybir.dt.bfloat16
F32 = mybir.dt.float32
F16 = mybir.dt.float16

P = 128      # partitions
T = 2048     # sequence length
D = 1024     # model dim
DH = 64      # head dim
NG = 4       # head pair-groups per core (2 heads each = 8 heads)
NCH = 4      # tq chunks of 512 per sequence
CH = 512     # tq chunk size
NTK = T // P  # 16 key tiles
ND = D // P   # 8 d-tiles
NE = D // P   # 8 e-tiles
N_CORES = 8

_CACHE = {}
import os
LAG = os.environ.get("NOLAG", "") == ""


def build_kernel(num_devices=N_CORES):
    nc = bacc.Bacc(num_devices=num_devices)

    xt = nc.dram_tensor("xt", [D, T], BF16, kind="ExternalInput")
    wq = nc.dram_tensor("wq", [D, NG * P], BF16, kind="ExternalInput")
    wk = nc.dram_tensor("wk", [D, NG * P], BF16, kind="ExternalInput")
    wv = nc.dram_tensor("wv", [D, NG * P], BF16, kind="ExternalInput")
    wp = nc.dram_tensor("wp", [D // 2, D], BF16, kind="ExternalInput")
    y = nc.dram_tensor("y", [D, T], F16, kind="ExternalOutput")

    with tile.TileContext(nc) as tc:
        with (
            tc.tile_pool(name="const", bufs=1) as cpool,
            tc.tile_pool(name="wpool", bufs=1) as wpool,
            tc.tile_pool(name="xpool", bufs=1) as xpool,
            tc.tile_pool(name="qkpool", bufs=2) as qkpool,
            tc.tile_pool(name="vpool", bufs=2) as vpool,
            tc.tile_pool(name="ptpool", bufs=6) as ptpool,
            tc.tile_pool(name="otpool", bufs=1) as otpool,
            tc.tile_pool(name="collpool", bufs=4) as collpool,
            tc.tile_pool(name="rpool", bufs=4) as rpool,
            tc.tile_pool(name="ypool", bufs=4) as ypool,
            tc.tile_pool(name="projpool", bufs=1) as projpool,
            tc.tile_pool(name="ps_s", bufs=2, space="PSUM") as ps_s,
            tc.tile_pool(name="ps_pv", bufs=2, space="PSUM") as ps_pv,
            tc.tile_pool(name="ps_acc", bufs=2, space="PSUM") as ps_acc,
        ):
            # ---- input DMAs: descriptor generation is ~0.6us per
            # dma_start and serializes per engine DGE, so spread the
            # issues across four engines' queues ----
            wk_sb = wpool.tile([P, ND, NG * P], BF16, tag="wk")
            nc.gpsimd.dma_start(wk_sb, wk.rearrange("(a p) b -> p a b", p=P))
            x_sb = []
            for d in range(ND):
                xd = xpool.tile([P, T], BF16, tag=f"x{d}")
                eng = (nc.sync, nc.gpsimd, nc.scalar)[d % 3]
                eng.dma_start(xd, xt[d * P:(d + 1) * P, :])
                x_sb.append(xd)
            wq_sb = wpool.tile([P, ND, NG * P], BF16, tag="wq")
            wv_sb = wpool.tile([P, ND, NG * P], BF16, tag="wv")
            nc.gpsimd.dma_start(wq_sb, wq.rearrange("(a p) b -> p a b", p=P))
            nc.scalar.dma_start(wv_sb, wv.rearrange("(a p) b -> p a b", p=P))
            wp_sb = projpool.tile([P, NG, D], BF16, tag="wp")
            nc.sync.dma_start(wp_sb, wp.rearrange("(a p) e -> p a e", p=P))

            ident = cpool.tile([P, P], BF16, tag="ident")
            make_identity(nc, ident)

            # attention output, transposed: [dh-pair(128), g*2048 + tq]
            ot_sb = otpool.tile([P, NG * T], BF16, tag="ot")

            qkv_tiles = {}  # g -> (qt, kt, vt, v_sb)

            def alloc_group(g):
                qkv_tiles[g] = (
                    qkpool.tile([P, T], BF16, tag="qt", name=f"qt{g}"),
                    qkpool.tile([P, T], BF16, tag="kt", name=f"kt{g}"),
                    qkpool.tile([P, T], BF16, tag="vt", name=f"vt{g}"),
                    vpool.tile([P, NTK * 130], BF16, tag="v", name=f"v{g}"),
                )
                nc.gpsimd.memset(qkv_tiles[g][3], 1.0)

            def qkv_gen(g):
                """One-op-at-a-time generator for group g's QKV + V
                transposes, ordered so the earliest-needed tensors finish
                first: K (all chunks), Q chunk 0, then V chunk c + its
                transposes, then Q chunks 1-3."""
                gc = slice(g * P, (g + 1) * P)
                vt, v_sb = qkv_tiles[g][2], qkv_tiles[g][3]

                def proj(which, c):
                    wsb = (wq_sb, wk_sb, wv_sb)[which]
                    dst = qkv_tiles[g][which]
                    psq = ps_acc.tile([P, CH], F32, tag="acc",
                                      name=f"qkv{g}_{which}_{c}")
                    for d in range(ND):
                        nc.tensor.matmul(
                            psq, lhsT=wsb[:, d, gc],
                            rhs=x_sb[d][:, c * CH:(c + 1) * CH],
                            start=(d == 0), stop=(d == ND - 1),
                        )
                        yield
                    nc.vector.tensor_copy(dst[:, c * CH:(c + 1) * CH], psq)
                    yield

                def vtr(tk):
                    pst = ps_acc.tile([P, P], BF16, tag="acc",
                                      name=f"vtr{g}_{tk}")
                    nc.tensor.transpose(pst, vt[:, tk * P:(tk + 1) * P], ident)
                    yield
                    nc.vector.tensor_copy(
                        v_sb[:, tk * 130:tk * 130 + 64], pst[:, 0:64]
                    )
                    nc.vector.tensor_copy(
                        v_sb[:, tk * 130 + 65:tk * 130 + 129], pst[:, 64:128]
                    )
                    yield

                for c in range(NCH):
                    yield from proj(1, c)      # K, all chunks
                yield from proj(0, 0)          # Q chunk 0
                for c in range(NCH):           # V chunk c + transposes
                    yield from proj(2, c)
                    for tk in range(4 * c, 4 * c + 4):
                        yield from vtr(tk)
                for c in range(1, NCH):        # Q chunks 1-3
                    yield from proj(0, c)

            # ops in qkv_gen: K 36, Q0 9, V+tr 4*(9+8)=68, Q1-3 27 = 140
            N_GEN_OPS = 140
            # eager prologue portion: everything except Q chunks 1-3
            N_EAGER = 113

            def make_feed(segments):
                """segments: list of [gen, n_ops, start_tile, end_tile).
                feed(t) advances each segment to its pro-rata target."""
                state = [[gen, n, s, e, 0] for gen, n, s, e in segments]

                def feed(t):
                    for seg in state:
                        gen, n, s, e, done = seg
                        if t < s:
                            continue
                        target = n if t >= e - 1 else (
                            n * (t - s + 1) + (e - s - 1)) // (e - s)
                        while seg[4] < target:
                            try:
                                next(gen)
                                seg[4] += 1
                            except StopIteration:
                                seg[4] = n
                                break
                return feed

            # ---- projection unit drip (group 3 window) ----
            proj_queue = []

            def proj_unit(e, ch):
                ec = slice(e * P, (e + 1) * P)
                psy = ps_acc.tile([P, CH], F32, tag="acc",
                                  name=f"y{e}_{ch}")
                for k in range(NG):
                    nc.tensor.matmul(
                        psy, lhsT=wp_sb[:, k, ec],
                        rhs=ot_sb[:, k * T + ch * CH:k * T + (ch + 1) * CH],
                        start=(k == 0), stop=(k == NG - 1),
                    )
                    yield
                ysb = ypool.tile([P, CH], F16, tag="ysb")
                nc.vector.tensor_copy(ysb, psy)
                yield
                nc.sync.dma_start(
                    y[e * P:(e + 1) * P, ch * CH:(ch + 1) * CH], ysb
                )
                yield

            def drain_proj(max_ops):
                done = 0
                while proj_queue and done < max_ops:
                    try:
                        next(proj_queue[0])
                        done += 1
                    except StopIteration:
                        proj_queue.pop(0)
                return done

            # ---- attention emission, software-pipelined one tile deep ----
            def make_pv(g, ch, tk, pt, pv0, pv1):
                v_sb = qkv_tiles[g][3]

                def emit():
                    nc.tensor.matmul(
                        pv0[0:65, :],
                        lhsT=v_sb[:, tk * 130:tk * 130 + 65],
                        rhs=pt[:, 0:CH],
                        start=(tk == 0), stop=(tk == NTK - 1),
                    )
                    nc.tensor.matmul(
                        pv1[0:65, :],
                        lhsT=v_sb[:, tk * 130 + 65:tk * 130 + 130],
                        rhs=pt[:, CH:2 * CH],
                        start=(tk == 0), stop=(tk == NTK - 1),
                    )
                return emit

            def make_finalize(g, ch, pv0, pv1):
                def emit():
                    # Drain the pv psums to SBUF at high priority so the
                    # copies jump the DVE queue and the psum slots free for
                    # the next chunk's accumulation.  Softmax denominators
                    # sit on row 64 of each pv psum; engine partition bases
                    # must be 32-aligned, so they land on rows 0 / 32 of the
                    # collector.
                    coll = collpool.tile([33, CH], F32, tag="coll",
                                         name=f"coll{g}_{ch}")
                    ost = []
                    with tc.high_priority():
                        # pv0's reads first so its psum slot frees for the
                        # next chunk's first accumulation one tile earlier
                        for h, pv in ((0, pv0), (1, pv1)):
                            nc.vector.tensor_copy(
                                coll[32 * h:32 * h + 1, :], pv[64:65, :]
                            )
                            o = collpool.tile([64, CH], F32, tag="ost",
                                              name=f"ost{g}_{ch}_{h}")
                            nc.vector.tensor_copy(o, pv[0:64, :])
                            ost.append(o)
                    # one exact reciprocal covers both rows (cost follows
                    # the free dim); rows 1-31 hold garbage and are unread.
                    rec = collpool.tile([33, CH], F32, tag="rec",
                                        name=f"rec{g}_{ch}")
                    nc.vector.reciprocal(rec, coll)
                    # the gpsimd partition_broadcast ucode reads partition 0
                    # of its source, so row 32 gets restaged at base 0.
                    rc1 = collpool.tile([1, CH], F32, tag="rc1",
                                        name=f"rc1_{g}_{ch}")
                    nc.vector.tensor_copy(rc1, rec[32:33, :])
                    cols = g * T + ch * CH
                    for h in range(2):
                        rb = rpool.tile([64, CH], F32, tag="rb")
                        nc.gpsimd.partition_broadcast(
                            rb, rec[0:1, :] if h == 0 else rc1
                        )
                        nc.vector.tensor_mul(
                            ot_sb[h * 64:(h + 1) * 64, cols:cols + CH],
                            ost[h], rb,
                        )
                    if g == NG - 1:
                        for e in range(NE):
                            proj_queue.append(proj_unit(e, ch))
                return emit

            # ---- prologue: all of group 0's K/Q-ch0/V/transposes ----
            alloc_group(0)
            g0gen = qkv_gen(0)
            for _ in range(N_EAGER):
                next(g0gen)

            # ---- main loop ----
            feeds = {}
            for g in range(NG):
                if g == 0:
                    alloc_group(1)
                    feeds[g] = make_feed([
                        [g0gen, N_GEN_OPS - N_EAGER, 0, 2 * NTK],
                        [qkv_gen(1), N_GEN_OPS, 0, NCH * NTK],
                    ])
                elif g < NG - 1:
                    alloc_group(g + 1)
                    feeds[g] = make_feed([
                        [qkv_gen(g + 1), N_GEN_OPS, 0, NCH * NTK],
                    ])
                else:
                    feeds[g] = lambda t: drain_proj(3)

            def emit_qk(g, ch, tk):
                qt, kt = qkv_tiles[g][0], qkv_tiles[g][1]
                ps = ps_s.tile([P, 2 * CH], F32, tag="s",
                               name=f"s{g}_{ch}_{tk}")
                nc.tensor.matmul(
                    ps[:, 0:CH],
                    lhsT=kt[0:64, tk * P:(tk + 1) * P],
                    rhs=qt[0:64, ch * CH:(ch + 1) * CH],
                    start=True, stop=True,
                )
                nc.tensor.matmul(
                    ps[:, CH:2 * CH],
                    lhsT=kt[64:128, tk * P:(tk + 1) * P],
                    rhs=qt[64:128, ch * CH:(ch + 1) * CH],
                    start=True, stop=True,
                )
                return ps

            iters = [(g, ch, tk) for g in range(NG) for ch in range(NCH)
                     for tk in range(NTK)]
            pv_tiles = {}
            carry = []
            ps_cur = emit_qk(0, 0, 0)
            for idx, (g, ch, tk) in enumerate(iters):
                if tk == 0:
                    pv_tiles[(g, ch)] = (
                        ps_pv.tile([P, CH], F32, tag="pv",
                                   name=f"pv0_{g}_{ch}"),
                        ps_pv.tile([P, CH], F32, tag="pv",
                                   name=f"pv1_{g}_{ch}"),
                    )
                pv0, pv1 = pv_tiles[(g, ch)]
                pt = ptpool.tile([P, 2 * CH], BF16, tag="pt")
                nc.scalar.activation(
                    pt, ps_cur, mybir.ActivationFunctionType.Exp,
                    scale=0.125,
                )
                # next tile's QK pair goes out right after the exp so it
                # outranks everything else pending on the PE
                if idx + 1 < len(iters):
                    gn, cn, tn = iters[idx + 1]
                    ps_cur = emit_qk(gn, cn, tn)
                for fn in carry:
                    fn()
                carry = [make_pv(g, ch, tk, pt, pv0, pv1)]
                if tk == NTK - 1:
                    carry.append(make_finalize(g, ch, pv0, pv1))
                feeds[g](ch * NTK + tk)

            # ---- tail: last pv + finalize + remaining projection units ----
            for fn in carry:
                fn()
            while proj_queue:
                drain_proj(1 << 30)

    nc.compile()
    return nc


def shard_inputs(x, w_qkv, w_proj):
    """Build the 8 per-core in_maps (host-side sharding + transposes)."""
    bf16 = ml_dtypes.bfloat16
    in_maps = []
    for c in range(N_CORES):
        b, s = divmod(c, 2)
        xt = np.ascontiguousarray(x[b].T).astype(bf16)  # [D, T]
        heads = [8 * s + 2 * g for g in range(NG)]

        def wslice(base):
            cols = [
                w_qkv[base + h * DH: base + (h + 2) * DH, :] for h in heads
            ]
            return np.ascontiguousarray(
                np.concatenate(cols, axis=0).T
            ).astype(bf16)

        rows = np.concatenate(
            [w_proj[:, (8 * s + 2 * g) * DH:(8 * s + 2 * g + 2) * DH].T
             for g in range(NG)], axis=0
        )
        in_maps.append({
            "xt": xt,
            "wq": wslice(0),
            "wk": wslice(D),
            "wv": wslice(2 * D),
            "wp": np.ascontiguousarray(rows).astype(bf16),
        })
    return in_maps


def assemble_output(results):
    out = np.empty((4, T, D), dtype=np.float32)
    for b in range(4):
        acc = (results[2 * b]["y"].astype(np.float32)
               + results[2 * b + 1]["y"].astype(np.float32))
        out[b] = acc.T
    return out


def run(x, w_qkv, w_proj, use_a2a=False, trace=False):
    del use_a2a
    if "k" not in _CACHE:
        _CACHE["k"] = build_kernel()
    nc = _CACHE["k"]
    in_maps = shard_inputs(x, w_qkv, w_proj)
    res = run_bass_kernel_spmd(
        nc, in_maps, core_ids=list(range(N_CORES)), trace=trace
    )
    return assemble_output(res.results), res


def kernel(x, w_qkv, w_proj):
    x = np.asarray(x, dtype=np.float32)
    w_qkv = np.asarray(w_qkv, dtype=np.float32)
    w_proj = np.asarray(w_proj, dtype=np.float32)
    out, _ = run(x, w_qkv, w_proj)
    return out
